# revision 2
# baseline (speedup 1.0000x reference)
"""Trainium2 kernel for nn_LoRALinear (moe_routing).

Math: reference computes out = x @ W.T + einsum('bri,bro->bo', a, b) with
a = A_table[dom].reshape(B,R,IN), b = B_table[dom].reshape(B,R,OUT).
The einsum contracts i over `a` alone, so the LoRA term collapses to a
per-domain table:
    L[d, o] = sum_r (sum_i A_table[d].reshape(R,IN)[r,i]) * B_table[d].reshape(R,OUT)[r,o]
    out = x @ W.T + L[domain_id]

On device this is a single augmented matmul per batch row:
    out[m, :] = [x[m, :], onehot(dom[m])] @ [[W.T], [L]]
with contraction K = 1024 + 64, evaluated as f32r matmuls (full rate on the
PE at N=512, ~fp32 precision) accumulated in PSUM.

Sharding: data-parallel over batch across 8 cores; the augmented weight is
replicated.
"""

import functools

import numpy as np

import concourse.mybir as mybir
import concourse.tile as tile
from concourse import bacc, bass_utils

B, D, R, ND = 16384, 1024, 8, 64
N_CORES = 8
BS = B // N_CORES            # 2048 batch rows per core
K_SIZES = [128] * 8 + [64]   # augmented contraction 1024 + 64
KA = sum(K_SIZES)
MB = 512                     # batch rows loaded per x DMA block
OH = 512                     # psum free dim (one bank)


@functools.lru_cache(maxsize=1)
def _build():
    nc = bacc.Bacc(None, target_bir_lowering=False, debug=False)
    xaT = nc.dram_tensor("xaT", [KA, BS], mybir.dt.float32, kind="ExternalInput")
    wa = nc.dram_tensor("wa", [KA, D], mybir.dt.float32, kind="ExternalInput")
    out = nc.dram_tensor("out", [BS, D], mybir.dt.float32, kind="ExternalOutput")
    f32r = mybir.dt.float32r

    with tile.TileContext(nc) as tc:
        with (
            tc.tile_pool(name="w", bufs=1) as wpool,
            tc.tile_pool(name="wraw", bufs=2) as wrawpool,
            tc.tile_pool(name="x", bufs=2) as xpool,
            tc.tile_pool(name="xraw", bufs=3) as xrawpool,
            tc.tile_pool(name="o", bufs=3) as opool,
            tc.tile_pool(name="ps", bufs=4, space="PSUM") as pspool,
        ):
            # fp32r matmul operands must be produced by a rounding op, so
            # every DMA lands in an f32 staging tile and a DVE copy rounds
            # it into the f32r tile the PE reads.
            wts = []
            k0 = 0
            for k, ks in enumerate(K_SIZES):
                wr = wrawpool.tile([ks, D], mybir.dt.float32, tag="wraw")
                nc.sync.dma_start(wr[:], wa[k0 : k0 + ks, :])
                wt = wpool.tile([ks, D], f32r, tag=f"w{k}")
                nc.vector.tensor_copy(wt[:], wr[:])
                wts.append(wt)
                k0 += ks

            for mb in range(BS // MB):
                xts = []
                k0 = 0
                for k, ks in enumerate(K_SIZES):
                    xr = xrawpool.tile([ks, MB], mybir.dt.float32, tag="xraw")
                    nc.sync.dma_start(
                        xr[:], xaT[k0 : k0 + ks, mb * MB : (mb + 1) * MB]
                    )
                    xt = xpool.tile([ks, MB], f32r, tag=f"x{k}")
                    nc.vector.tensor_copy(xt[:], xr[:])
                    xts.append(xt)
                    k0 += ks

                for mt in range(MB // 128):
                    ot = opool.tile([128, D], mybir.dt.float32, tag="ot")
                    for oh in range(D // OH):
                        ps = pspool.tile([128, OH], mybir.dt.float32, tag="ps")
                        for k in range(len(K_SIZES)):
                            nc.tensor.matmul(
                                ps[:],
                                xts[k][:, mt * 128 : (mt + 1) * 128],
                                wts[k][:, oh * OH : (oh + 1) * OH],
                                start=(k == 0),
                                stop=(k == len(K_SIZES) - 1),
                            )
                        nc.scalar.copy(ot[:, oh * OH : (oh + 1) * OH], ps[:])
                    m0 = mb * MB + mt * 128
                    nc.sync.dma_start(out[m0 : m0 + 128, :], ot[:])

    nc.compile()
    return nc


def _prepare(x, W, A_table, B_table, domain_id):
    x = np.asarray(x, dtype=np.float32)
    W = np.asarray(W, dtype=np.float32)
    A = np.asarray(A_table, dtype=np.float64)
    Bt = np.asarray(B_table, dtype=np.float64)
    dom = np.asarray(domain_id).astype(np.int64)

    sA = A.reshape(ND, R, D).sum(axis=2)                        # [ND, R]
    L = np.einsum("dr,dro->do", sA, Bt.reshape(ND, R, D))       # [ND, D]

    wa = np.empty((KA, D), dtype=np.float32)
    wa[:D] = W.T
    wa[D:] = L.astype(np.float32)

    in_maps = []
    for c in range(N_CORES):
        sl = slice(c * BS, (c + 1) * BS)
        xaT_c = np.empty((KA, BS), dtype=np.float32)
        xaT_c[:D] = x[sl].T
        xaT_c[D:] = np.arange(ND, dtype=np.int64)[:, None] == dom[None, sl]
        in_maps.append({"xaT": xaT_c, "wa": wa})
    return in_maps


def kernel(x, W, A_table, B_table, domain_id, _trace=False):
    in_maps = _prepare(x, W, A_table, B_table, domain_id)
    nc = _build()
    res = bass_utils.run_bass_kernel_spmd(
        nc, in_maps, core_ids=list(range(N_CORES)), trace=_trace
    )
    out = np.concatenate([res.results[c]["out"] for c in range(N_CORES)], axis=0)
    if _trace:
        kernel.last_results = res
    return out


# revision 3
# speedup vs baseline: 1.2218x; 1.2218x over previous
"""Trainium2 kernel for nn_LoRALinear (moe_routing).

Math: reference computes out = x @ W.T + einsum('bri,bro->bo', a, b) with
a = A_table[dom].reshape(B,R,IN), b = B_table[dom].reshape(B,R,OUT).
The einsum contracts i over `a` alone, so the LoRA term collapses to a
per-domain table:
    L[d, o] = sum_r (sum_i A_table[d].reshape(R,IN)[r,i]) * B_table[d].reshape(R,OUT)[r,o]
    out = x @ W.T + L[domain_id]

On device this is a single augmented matmul per batch row:
    out[m, :] = [x[m, :], onehot(dom[m])] @ [[W.T], [L]]
with contraction K = 1024 + 64, accumulated in PSUM. The one-hot rows
select L rows exactly (0/1 are exact in bf16).

Sharding: data-parallel over batch across 8 cores; the augmented weight is
replicated.
"""

import functools

import numpy as np

import concourse.mybir as mybir
import concourse.tile as tile
from concourse import bacc, bass_utils

B, D, R, ND = 16384, 1024, 8, 64
N_CORES = 8
BS = B // N_CORES            # 2048 batch rows per core
K_SIZES = [128] * 8 + [64]   # augmented contraction 1024 + 64
KA = sum(K_SIZES)
MB = 512                     # batch rows loaded per x DMA block
OH = 512                     # psum free dim (one bank)

PRECISION = "bf16"           # "bf16" (1 cyc/row PE) or "f32r" (2 cyc/row)


def _np_in_dtype():
    if PRECISION == "bf16":
        import ml_dtypes

        return np.dtype(ml_dtypes.bfloat16)
    return np.dtype(np.float32)


@functools.lru_cache(maxsize=1)
def _build():
    nc = bacc.Bacc(None, target_bir_lowering=False, debug=False)
    in_dt = mybir.dt.bfloat16 if PRECISION == "bf16" else mybir.dt.float32
    xaT = nc.dram_tensor("xaT", [KA, BS], in_dt, kind="ExternalInput")
    wa = nc.dram_tensor("wa", [KA, D], in_dt, kind="ExternalInput")
    out = nc.dram_tensor("out", [BS, D], mybir.dt.float32, kind="ExternalOutput")
    f32r = mybir.dt.float32r

    with tile.TileContext(nc) as tc:
        with (
            tc.tile_pool(name="w", bufs=1) as wpool,
            tc.tile_pool(name="wraw", bufs=2) as wrawpool,
            tc.tile_pool(name="x", bufs=2) as xpool,
            tc.tile_pool(name="xraw", bufs=3) as xrawpool,
            tc.tile_pool(name="o", bufs=3) as opool,
            tc.tile_pool(name="ps", bufs=4, space="PSUM") as pspool,
        ):
            wts = []
            k0 = 0
            for k, ks in enumerate(K_SIZES):
                if PRECISION == "bf16":
                    wt = wpool.tile([ks, D], in_dt, tag=f"w{k}")
                    nc.sync.dma_start(wt[:], wa[k0 : k0 + ks, :])
                else:
                    # fp32r matmul operands must be produced by a rounding
                    # op: DMA to f32 staging, DVE copy rounds into f32r.
                    wr = wrawpool.tile([ks, D], mybir.dt.float32, tag="wraw")
                    nc.sync.dma_start(wr[:], wa[k0 : k0 + ks, :])
                    wt = wpool.tile([ks, D], f32r, tag=f"w{k}")
                    nc.vector.tensor_copy(wt[:], wr[:])
                wts.append(wt)
                k0 += ks

            for mb in range(BS // MB):
                xts = []
                k0 = 0
                for k, ks in enumerate(K_SIZES):
                    if PRECISION == "bf16":
                        xt = xpool.tile([ks, MB], in_dt, tag=f"x{k}")
                        nc.sync.dma_start(
                            xt[:], xaT[k0 : k0 + ks, mb * MB : (mb + 1) * MB]
                        )
                    else:
                        xr = xrawpool.tile([ks, MB], mybir.dt.float32, tag="xraw")
                        nc.sync.dma_start(
                            xr[:], xaT[k0 : k0 + ks, mb * MB : (mb + 1) * MB]
                        )
                        xt = xpool.tile([ks, MB], f32r, tag=f"x{k}")
                        nc.vector.tensor_copy(xt[:], xr[:])
                    xts.append(xt)
                    k0 += ks

                for mt in range(MB // 128):
                    ot = opool.tile([128, D], mybir.dt.float32, tag="ot")
                    for oh in range(D // OH):
                        ps = pspool.tile([128, OH], mybir.dt.float32, tag="ps")
                        for k in range(len(K_SIZES)):
                            nc.tensor.matmul(
                                ps[:],
                                xts[k][:, mt * 128 : (mt + 1) * 128],
                                wts[k][:, oh * OH : (oh + 1) * OH],
                                start=(k == 0),
                                stop=(k == len(K_SIZES) - 1),
                            )
                        nc.vector.tensor_copy(ot[:, oh * OH : (oh + 1) * OH], ps[:])
                    m0 = mb * MB + mt * 128
                    nc.sync.dma_start(out[m0 : m0 + 128, :], ot[:])

    nc.compile()
    return nc


def _prepare(x, W, A_table, B_table, domain_id):
    x = np.asarray(x, dtype=np.float32)
    W = np.asarray(W, dtype=np.float32)
    A = np.asarray(A_table, dtype=np.float64)
    Bt = np.asarray(B_table, dtype=np.float64)
    dom = np.asarray(domain_id).astype(np.int64)

    sA = A.reshape(ND, R, D).sum(axis=2)                        # [ND, R]
    L = np.einsum("dr,dro->do", sA, Bt.reshape(ND, R, D))       # [ND, D]

    in_dt = _np_in_dtype()
    wa = np.empty((KA, D), dtype=in_dt)
    wa[:D] = W.T.astype(in_dt)
    wa[D:] = L.astype(np.float32).astype(in_dt)

    in_maps = []
    xT = np.ascontiguousarray(x.T).astype(in_dt)                # [D, B]
    onehotT = (
        np.arange(ND, dtype=np.int64)[:, None] == dom[None, :]
    ).astype(in_dt)                                             # [ND, B]
    for c in range(N_CORES):
        sl = slice(c * BS, (c + 1) * BS)
        xaT_c = np.empty((KA, BS), dtype=in_dt)
        xaT_c[:D] = xT[:, sl]
        xaT_c[D:] = onehotT[:, sl]
        in_maps.append({"xaT": xaT_c, "wa": wa})
    return in_maps


def kernel(x, W, A_table, B_table, domain_id, _trace=False):
    in_maps = _prepare(x, W, A_table, B_table, domain_id)
    nc = _build()
    res = bass_utils.run_bass_kernel_spmd(
        nc, in_maps, core_ids=list(range(N_CORES)), trace=_trace
    )
    out = np.concatenate([res.results[c]["out"] for c in range(N_CORES)], axis=0)
    if _trace:
        kernel.last_results = res
    return out


# revision 6
# speedup vs baseline: 1.3318x; 1.0901x over previous
"""Trainium2 kernel for nn_LoRALinear (moe_routing).

Math: reference computes out = x @ W.T + einsum('bri,bro->bo', a, b) with
a = A_table[dom].reshape(B,R,IN), b = B_table[dom].reshape(B,R,OUT).
The einsum contracts i over `a` alone, so the LoRA term collapses to a
per-domain table:
    L[d, o] = sum_r (sum_i A_table[d].reshape(R,IN)[r,i]) * B_table[d].reshape(R,OUT)[r,o]
    out = x @ W.T + L[domain_id]

On device this is a single augmented matmul per batch row:
    out[m, :] = [x[m, :], onehot(dom[m])] @ [[W.T], [L]]
with contraction K = 1024 + 64 (padded to 1152 = 9*128), accumulated in
PSUM. The one-hot rows select L rows exactly (0/1 are exact in bf16).

Sharding: data-parallel over batch across 8 cores; the augmented weight is
replicated.

Device layout: the host pre-transposes activations into chunk-major form
xa[p, mb, k, j] = xaT[k*128 + p, mb*MB + j] so each m-block is a single
contiguous-per-partition DMA covering all 9 K-chunks.
"""

import functools

import numpy as np

import concourse.mybir as mybir
import concourse.tile as tile
from concourse import bacc, bass_utils

B, D, R, ND = 16384, 1024, 8, 64
N_CORES = 8
BS = B // N_CORES            # 2048 batch rows per core
NK = 9                       # K chunks of 128 (1024 + 64 padded to 1152)
KA = NK * 128
MB = 512                     # batch rows per x block DMA
NMB = BS // MB               # 4 blocks
OH = 512                     # psum free dim (one bank)


@functools.lru_cache(maxsize=1)
def _build():
    nc = bacc.Bacc(None, target_bir_lowering=False, debug=False)
    bf16 = mybir.dt.bfloat16
    xa = nc.dram_tensor("xa", [128, NMB * NK * MB], bf16, kind="ExternalInput")
    wa = nc.dram_tensor("wa", [KA, D], bf16, kind="ExternalInput")
    out = nc.dram_tensor("out", [BS, D], mybir.dt.float32, kind="ExternalOutput")

    with tile.TileContext(nc) as tc:
        with (
            tc.tile_pool(name="w", bufs=1) as wpool,
            tc.tile_pool(name="x", bufs=2) as xpool,
            tc.tile_pool(name="o", bufs=4) as opool,
            tc.tile_pool(name="ps", bufs=8, space="PSUM") as pspool,
        ):
            # x block 0 first so its transfer overlaps the W preload.
            xts = {}
            xt0 = xpool.tile([128, NK * MB], bf16, tag="x")
            nc.sync.dma_start(xt0[:], xa[:, 0 : NK * MB])
            xts[0] = xt0

            wts = []
            for k in range(NK):
                wt = wpool.tile([128, D], bf16, tag=f"w{k}")
                nc.sync.dma_start(wt[:], wa[k * 128 : (k + 1) * 128, :])
                wts.append(wt)

            for mb in range(NMB):
                if mb not in xts:
                    xtn = xpool.tile([128, NK * MB], bf16, tag="x")
                    nc.sync.dma_start(
                        xtn[:], xa[:, mb * NK * MB : (mb + 1) * NK * MB]
                    )
                    xts[mb] = xtn
                xt = xts[mb]
                for mt in range(MB // 128):
                    ot = opool.tile([128, D], mybir.dt.float32, tag="ot")
                    for oh in range(D // OH):
                        ps = pspool.tile([128, OH], mybir.dt.float32, tag="ps")
                        for k in range(NK):
                            nc.tensor.matmul(
                                ps[:],
                                xt[:, k * MB + mt * 128 : k * MB + (mt + 1) * 128],
                                wts[k][:, oh * OH : (oh + 1) * OH],
                                start=(k == 0),
                                stop=(k == NK - 1),
                            )
                        nc.vector.tensor_copy(ot[:, oh * OH : (oh + 1) * OH], ps[:])
                    m0 = mb * MB + mt * 128
                    nc.sync.dma_start(out[m0 : m0 + 128, :], ot[:])

    nc.compile()
    return nc


def _prepare(x, W, A_table, B_table, domain_id):
    import ml_dtypes

    bf16 = np.dtype(ml_dtypes.bfloat16)
    x = np.asarray(x, dtype=np.float32)
    W = np.asarray(W, dtype=np.float32)
    A = np.asarray(A_table, dtype=np.float64)
    Bt = np.asarray(B_table, dtype=np.float64)
    dom = np.asarray(domain_id).astype(np.int64)

    sA = A.reshape(ND, R, D).sum(axis=2)                        # [ND, R]
    L = np.einsum("dr,dro->do", sA, Bt.reshape(ND, R, D))       # [ND, D]

    wa = np.zeros((KA, D), dtype=bf16)
    wa[:D] = W.T.astype(bf16)
    wa[D : D + ND] = L.astype(np.float32).astype(bf16)

    xT = np.ascontiguousarray(x.T).astype(bf16)                 # [D, B]
    onehotT = (
        np.arange(ND, dtype=np.int64)[:, None] == dom[None, :]
    ).astype(bf16)                                              # [ND, B]

    in_maps = []
    for c in range(N_CORES):
        sl = slice(c * BS, (c + 1) * BS)
        xaT_c = np.zeros((KA, BS), dtype=bf16)
        xaT_c[:D] = xT[:, sl]
        xaT_c[D : D + ND] = onehotT[:, sl]
        # chunk-major: xa[p, mb, k, j] = xaT_c[k*128 + p, mb*MB + j]
        xa_c = np.ascontiguousarray(
            xaT_c.reshape(NK, 128, NMB, MB).transpose(1, 2, 0, 3)
        ).reshape(128, NMB * NK * MB)
        in_maps.append({"xa": xa_c, "wa": wa})
    return in_maps


def kernel(x, W, A_table, B_table, domain_id, _trace=False):
    in_maps = _prepare(x, W, A_table, B_table, domain_id)
    nc = _build()
    res = bass_utils.run_bass_kernel_spmd(
        nc, in_maps, core_ids=list(range(N_CORES)), trace=_trace
    )
    out = np.concatenate([res.results[c]["out"] for c in range(N_CORES)], axis=0)
    if _trace:
        kernel.last_results = res
    return out


# revision 7
# speedup vs baseline: 1.3559x; 1.0181x over previous
"""Trainium2 kernel for nn_LoRALinear (moe_routing).

Math: reference computes out = x @ W.T + einsum('bri,bro->bo', a, b) with
a = A_table[dom].reshape(B,R,IN), b = B_table[dom].reshape(B,R,OUT).
The einsum contracts i over `a` alone, so the LoRA term collapses to a
per-domain table:
    L[d, o] = sum_r (sum_i A_table[d].reshape(R,IN)[r,i]) * B_table[d].reshape(R,OUT)[r,o]
    out = x @ W.T + L[domain_id]

On device this is a single augmented matmul per batch row:
    out[m, :] = [x[m, :], onehot(dom[m])] @ [[W.T], [L]]
with contraction K = 1024 (8 chunks of 128) plus a K=64 one-hot chunk.
The one-hot rows select L rows exactly (0/1 are exact in bf16). The two
K=64 one-hot matmuls per m-tile are packed into disjoint PE row groups
(tile_position) so they run concurrently.

Sharding: data-parallel over batch across 8 cores; the augmented weight is
replicated.

Device layout: the host pre-transposes activations into chunk-major form
xa[p, mb, k, j] = xaT[k*128 + p, mb*MB + j] so each m-block is a single
contiguous-per-partition DMA covering all 9 K-chunks (chunk 8 carries the
one-hot rows duplicated into both half-partitions).
"""

import functools

import numpy as np

import concourse.mybir as mybir
import concourse.tile as tile
from concourse import bacc, bass_utils

B, D, R, ND = 16384, 1024, 8, 64
N_CORES = 8
BS = B // N_CORES            # 2048 batch rows per core
NKW = 8                      # K chunks of 128 for the dense W part
NK = NKW + 1                 # + one-hot chunk
MB = 512                     # batch rows per x block DMA
NMB = BS // MB               # 4 blocks
OH = 512                     # psum free dim (one bank)


@functools.lru_cache(maxsize=1)
def _build():
    nc = bacc.Bacc(None, target_bir_lowering=False, debug=False)
    bf16 = mybir.dt.bfloat16
    xa = nc.dram_tensor("xa", [128, NMB * NK * MB], bf16, kind="ExternalInput")
    wa = nc.dram_tensor("wa", [NKW * 128, D], bf16, kind="ExternalInput")
    # L table packed for row-group concurrency: rows 0:64 = L[:, 0:512],
    # rows 64:128 = L[:, 512:1024]
    w8 = nc.dram_tensor("w8", [128, OH], bf16, kind="ExternalInput")
    out = nc.dram_tensor("out", [BS, D], mybir.dt.float32, kind="ExternalOutput")

    with tile.TileContext(nc) as tc:
        with (
            tc.tile_pool(name="w", bufs=1) as wpool,
            tc.tile_pool(name="x", bufs=2) as xpool,
            tc.tile_pool(name="o", bufs=4) as opool,
            tc.tile_pool(name="ps", bufs=8, space="PSUM") as pspool,
        ):
            # x block 0 first so its transfer overlaps the W preload.
            xts = {}
            xt0 = xpool.tile([128, NK * MB], bf16, tag="x")
            nc.sync.dma_start(xt0[:], xa[:, 0 : NK * MB])
            xts[0] = xt0

            wts = []
            for k in range(NKW):
                wt = wpool.tile([128, D], bf16, tag=f"w{k}")
                nc.sync.dma_start(wt[:], wa[k * 128 : (k + 1) * 128, :])
                wts.append(wt)
            w8t = wpool.tile([128, OH], bf16, tag="w8")
            nc.sync.dma_start(w8t[:], w8[:, :])

            def xsl(xt, k, mt):
                return xt[:, k * MB + mt * 128 : k * MB + (mt + 1) * 128]

            def finish(xt, mt, pss, mb):
                """One-hot row-group-packed matmuls + psum copies + out DMA."""
                nc.tensor.matmul(
                    pss[0][:],
                    xt[0:64, NKW * MB + mt * 128 : NKW * MB + (mt + 1) * 128],
                    w8t[0:64, :],
                    start=False,
                    stop=True,
                    tile_position=(0, 0),
                )
                nc.tensor.matmul(
                    pss[1][:],
                    xt[64:128, NKW * MB + mt * 128 : NKW * MB + (mt + 1) * 128],
                    w8t[64:128, :],
                    start=False,
                    stop=True,
                    tile_position=(64, 0),
                )
                ot = opool.tile([128, D], mybir.dt.float32, tag="ot")
                nc.vector.tensor_copy(ot[:, 0:OH], pss[0][:])
                nc.vector.tensor_copy(ot[:, OH : 2 * OH], pss[1][:])
                m0 = mb * MB + mt * 128
                nc.sync.dma_start(out[m0 : m0 + 128, :], ot[:])

            # First two m-tiles: k-interleaved across 4 psum groups so each
            # arriving W chunk immediately feeds 4 matmuls (keeps the PE fed
            # while W streams in).
            pss = {}
            for g in range(4):
                psg = pspool.tile([128, OH], mybir.dt.float32, tag="ps")
                pss[g] = psg
            for k in range(NKW):
                for g in range(4):
                    mt, oh = divmod(g, 2)
                    nc.tensor.matmul(
                        pss[g][:],
                        xsl(xt0, k, mt),
                        wts[k][:, oh * OH : (oh + 1) * OH],
                        start=(k == 0),
                        stop=False,
                    )
            finish(xt0, 0, (pss[0], pss[1]), 0)
            finish(xt0, 1, (pss[2], pss[3]), 0)

            for mb in range(NMB):
                if mb not in xts:
                    xtn = xpool.tile([128, NK * MB], bf16, tag="x")
                    nc.sync.dma_start(
                        xtn[:], xa[:, mb * NK * MB : (mb + 1) * NK * MB]
                    )
                    xts[mb] = xtn
                xt = xts[mb]
                for mt in range(MB // 128):
                    if mb == 0 and mt < 2:
                        continue  # handled by the k-interleaved prologue
                    ps0 = pspool.tile([128, OH], mybir.dt.float32, tag="ps")
                    ps1 = pspool.tile([128, OH], mybir.dt.float32, tag="ps")
                    for k in range(NKW):
                        nc.tensor.matmul(
                            ps0[:],
                            xsl(xt, k, mt),
                            wts[k][:, 0:OH],
                            start=(k == 0),
                            stop=False,
                        )
                    for k in range(NKW):
                        nc.tensor.matmul(
                            ps1[:],
                            xsl(xt, k, mt),
                            wts[k][:, OH : 2 * OH],
                            start=(k == 0),
                            stop=False,
                        )
                    finish(xt, mt, (ps0, ps1), mb)

    nc.compile()
    return nc


def _prepare(x, W, A_table, B_table, domain_id):
    import ml_dtypes

    bf16 = np.dtype(ml_dtypes.bfloat16)
    x = np.asarray(x, dtype=np.float32)
    W = np.asarray(W, dtype=np.float32)
    A = np.asarray(A_table, dtype=np.float64)
    Bt = np.asarray(B_table, dtype=np.float64)
    dom = np.asarray(domain_id).astype(np.int64)

    sA = A.reshape(ND, R, D).sum(axis=2)                        # [ND, R]
    L = np.einsum("dr,dro->do", sA, Bt.reshape(ND, R, D))       # [ND, D]
    Lb = L.astype(np.float32).astype(bf16)

    wa = np.ascontiguousarray(W.T.astype(bf16))                 # [D, D]
    w8 = np.empty((128, OH), dtype=bf16)
    w8[0:ND] = Lb[:, 0:OH]
    w8[ND : 2 * ND] = Lb[:, OH : 2 * OH]

    xT = np.ascontiguousarray(x.T).astype(bf16)                 # [D, B]
    onehotT = (
        np.arange(ND, dtype=np.int64)[:, None] == dom[None, :]
    ).astype(bf16)                                              # [ND, B]

    in_maps = []
    for c in range(N_CORES):
        sl = slice(c * BS, (c + 1) * BS)
        xaT_c = np.empty((NK * 128, BS), dtype=bf16)
        xaT_c[: NKW * 128] = xT[:, sl]
        xaT_c[NKW * 128 : NKW * 128 + ND] = onehotT[:, sl]
        xaT_c[NKW * 128 + ND :] = onehotT[:, sl]                # duplicate
        # chunk-major: xa[p, mb, k, j] = xaT_c[k*128 + p, mb*MB + j]
        xa_c = np.ascontiguousarray(
            xaT_c.reshape(NK, 128, NMB, MB).transpose(1, 2, 0, 3)
        ).reshape(128, NMB * NK * MB)
        in_maps.append({"xa": xa_c, "wa": wa, "w8": w8})
    return in_maps


def kernel(x, W, A_table, B_table, domain_id, _trace=False):
    in_maps = _prepare(x, W, A_table, B_table, domain_id)
    nc = _build()
    res = bass_utils.run_bass_kernel_spmd(
        nc, in_maps, core_ids=list(range(N_CORES)), trace=_trace
    )
    out = np.concatenate([res.results[c]["out"] for c in range(N_CORES)], axis=0)
    if _trace:
        kernel.last_results = res
    return out


# revision 11
# speedup vs baseline: 1.4130x; 1.0421x over previous
"""Trainium2 kernel for nn_LoRALinear (moe_routing).

Math: reference computes out = x @ W.T + einsum('bri,bro->bo', a, b) with
a = A_table[dom].reshape(B,R,IN), b = B_table[dom].reshape(B,R,OUT).
The einsum contracts i over `a` alone, so the LoRA term collapses to a
per-domain table:
    L[d, o] = sum_r (sum_i A_table[d].reshape(R,IN)[r,i]) * B_table[d].reshape(R,OUT)[r,o]
    out = x @ W.T + L[domain_id]

On device this is a single augmented matmul per batch row:
    out[m, :] = [x[m, :], onehot(dom[m])] @ [[W.T], [L]]
with contraction K = 1024 (8 chunks of 128) plus a K=64 one-hot chunk.
The one-hot rows select L rows exactly (0/1 are exact in bf16). The two
K=64 one-hot matmuls per m-tile are packed into disjoint PE row groups
(tile_position) so they run concurrently.

Sharding: data-parallel over batch across 8 cores; the augmented weight is
replicated.

Device layout: the host pre-transposes activations into chunk-major form
xa[p, mb, k, j] = xaT[k*128 + p, mb*MB + j] so each m-block is a single
contiguous-per-partition DMA covering all 9 K-chunks (chunk 8 carries the
one-hot rows duplicated into both half-partitions).
"""

import functools

import numpy as np

import concourse.mybir as mybir
import concourse.tile as tile
from concourse import bacc, bass_utils

B, D, R, ND = 16384, 1024, 8, 64
N_CORES = 8
BS = B // N_CORES            # 2048 batch rows per core
NKW = 8                      # K chunks of 128 for the dense W part
NK = NKW + 1                 # + one-hot chunk
MB = 512                     # batch rows per x block DMA
NMB = BS // MB               # 4 blocks
OH = 512                     # psum free dim (one bank)


@functools.lru_cache(maxsize=1)
def _build():
    nc = bacc.Bacc(None, target_bir_lowering=False, debug=False)
    bf16 = mybir.dt.bfloat16
    xa = nc.dram_tensor("xa", [128, NMB * NK * MB], bf16, kind="ExternalInput")
    wa = nc.dram_tensor("wa", [NKW * 128, D], bf16, kind="ExternalInput")
    # L table packed for row-group concurrency: rows 0:64 = L[:, 0:512],
    # rows 64:128 = L[:, 512:1024]
    w8 = nc.dram_tensor("w8", [128, OH], bf16, kind="ExternalInput")
    out = nc.dram_tensor("out", [BS, D], mybir.dt.float32, kind="ExternalOutput")

    with tile.TileContext(nc) as tc:
        with (
            tc.tile_pool(name="w", bufs=1) as wpool,
            tc.tile_pool(name="x", bufs=2) as xpool,
            tc.tile_pool(name="o", bufs=4) as opool,
            tc.tile_pool(name="ps", bufs=7, space="PSUM") as pspool,
            tc.tile_pool(name="dps", bufs=1, space="PSUM") as dpspool,
        ):
            # Warm the PE (HAM clock gate) with dummy matmuls on a scratch
            # tile while the first DMAs stream in; otherwise the first ~12
            # real matmuls run at half clock.
            scratch = wpool.tile([128, OH], bf16, tag="scratch")
            nc.gpsimd.memset(scratch[:], 0.0)
            dps = dpspool.tile([128, OH], mybir.dt.float32, tag="dps")
            for i in range(12):
                nc.tensor.matmul(
                    dps[:],
                    scratch[:, 0:128],
                    scratch[:],
                    start=(i == 0),
                    stop=(i == 11),
                )

            # x block 0 first so its transfer overlaps the W preload.
            xts = {}
            xt0 = xpool.tile([128, NK * MB], bf16, tag="x")
            nc.sync.dma_start(xt0[:], xa[:, 0 : NK * MB])
            xts[0] = xt0

            wts = []
            for k in range(NKW):
                wt = wpool.tile([128, D], bf16, tag=f"w{k}")
                nc.sync.dma_start(wt[:], wa[k * 128 : (k + 1) * 128, :])
                wts.append(wt)
            w8t = wpool.tile([128, OH], bf16, tag="w8")
            nc.sync.dma_start(w8t[:], w8[:, :])

            def xsl(xt, k, mt):
                return xt[:, k * MB + mt * 128 : k * MB + (mt + 1) * 128]

            def finish(xt, mt, pss, mb):
                """One-hot row-group-packed matmuls + psum copies + out DMA."""
                nc.tensor.matmul(
                    pss[0][:],
                    xt[0:64, NKW * MB + mt * 128 : NKW * MB + (mt + 1) * 128],
                    w8t[0:64, :],
                    start=False,
                    stop=True,
                    tile_position=(0, 0),
                )
                nc.tensor.matmul(
                    pss[1][:],
                    xt[64:128, NKW * MB + mt * 128 : NKW * MB + (mt + 1) * 128],
                    w8t[64:128, :],
                    start=False,
                    stop=True,
                    tile_position=(64, 0),
                )
                ot = opool.tile([128, D], mybir.dt.float32, tag="ot")
                nc.vector.tensor_copy(ot[:, 0:OH], pss[0][:])
                nc.scalar.copy(ot[:, OH : 2 * OH], pss[1][:])
                m0 = mb * MB + mt * 128
                nc.sync.dma_start(out[m0 : m0 + 128, :], ot[:])

            # First two m-tiles: k-interleaved across 4 psum groups so each
            # arriving W chunk immediately feeds 4 matmuls (keeps the PE fed
            # while W streams in).
            pss = {}
            for g in range(4):
                psg = pspool.tile([128, OH], mybir.dt.float32, tag="ps")
                pss[g] = psg
            for k in range(NKW):
                for g in range(4):
                    mt, oh = divmod(g, 2)
                    nc.tensor.matmul(
                        pss[g][:],
                        xsl(xt0, k, mt),
                        wts[k][:, oh * OH : (oh + 1) * OH],
                        start=(k == 0),
                        stop=False,
                    )
            finish(xt0, 0, (pss[0], pss[1]), 0)
            finish(xt0, 1, (pss[2], pss[3]), 0)

            for mb in range(NMB):
                if mb not in xts:
                    xtn = xpool.tile([128, NK * MB], bf16, tag="x")
                    nc.sync.dma_start(
                        xtn[:], xa[:, mb * NK * MB : (mb + 1) * NK * MB]
                    )
                    xts[mb] = xtn
                xt = xts[mb]
                for mt in range(MB // 128):
                    if mb == 0 and mt < 2:
                        continue  # handled by the k-interleaved prologue
                    ps0 = pspool.tile([128, OH], mybir.dt.float32, tag="ps")
                    ps1 = pspool.tile([128, OH], mybir.dt.float32, tag="ps")
                    for k in range(NKW):
                        nc.tensor.matmul(
                            ps0[:],
                            xsl(xt, k, mt),
                            wts[k][:, 0:OH],
                            start=(k == 0),
                            stop=False,
                        )
                    for k in range(NKW):
                        nc.tensor.matmul(
                            ps1[:],
                            xsl(xt, k, mt),
                            wts[k][:, OH : 2 * OH],
                            start=(k == 0),
                            stop=False,
                        )
                    finish(xt, mt, (ps0, ps1), mb)

    nc.compile()
    return nc


def _prepare(x, W, A_table, B_table, domain_id):
    import ml_dtypes

    bf16 = np.dtype(ml_dtypes.bfloat16)
    x = np.asarray(x, dtype=np.float32)
    W = np.asarray(W, dtype=np.float32)
    A = np.asarray(A_table, dtype=np.float64)
    Bt = np.asarray(B_table, dtype=np.float64)
    dom = np.asarray(domain_id).astype(np.int64)

    sA = A.reshape(ND, R, D).sum(axis=2)                        # [ND, R]
    L = np.einsum("dr,dro->do", sA, Bt.reshape(ND, R, D))       # [ND, D]
    Lb = L.astype(np.float32).astype(bf16)

    wa = np.ascontiguousarray(W.T.astype(bf16))                 # [D, D]
    w8 = np.empty((128, OH), dtype=bf16)
    w8[0:ND] = Lb[:, 0:OH]
    w8[ND : 2 * ND] = Lb[:, OH : 2 * OH]

    xT = np.ascontiguousarray(x.T).astype(bf16)                 # [D, B]
    onehotT = (
        np.arange(ND, dtype=np.int64)[:, None] == dom[None, :]
    ).astype(bf16)                                              # [ND, B]

    in_maps = []
    for c in range(N_CORES):
        sl = slice(c * BS, (c + 1) * BS)
        xaT_c = np.empty((NK * 128, BS), dtype=bf16)
        xaT_c[: NKW * 128] = xT[:, sl]
        xaT_c[NKW * 128 : NKW * 128 + ND] = onehotT[:, sl]
        xaT_c[NKW * 128 + ND :] = onehotT[:, sl]                # duplicate
        # chunk-major: xa[p, mb, k, j] = xaT_c[k*128 + p, mb*MB + j]
        xa_c = np.ascontiguousarray(
            xaT_c.reshape(NK, 128, NMB, MB).transpose(1, 2, 0, 3)
        ).reshape(128, NMB * NK * MB)
        in_maps.append({"xa": xa_c, "wa": wa, "w8": w8})
    return in_maps


def kernel(x, W, A_table, B_table, domain_id, _trace=False):
    in_maps = _prepare(x, W, A_table, B_table, domain_id)
    nc = _build()
    res = bass_utils.run_bass_kernel_spmd(
        nc, in_maps, core_ids=list(range(N_CORES)), trace=_trace
    )
    out = np.concatenate([res.results[c]["out"] for c in range(N_CORES)], axis=0)
    if _trace:
        kernel.last_results = res
    return out
